# revision 22
# baseline (speedup 1.0000x reference)
"""KD feature-level smooth-L1 loss kernel for Trainium2 (8 NeuronCores).

Math (per batch sample b over (C,H,W) = 256*64*64 = N elements):
  t_norm = (t - mean) * rsqrt(var + eps)          # LayerNorm, no affine
  d   = |t_norm - s|
  kd  = where(d <= 2, d*d/4, d - 1)               # smooth-L1, beta=2
  out = mean_b( sum_chw(kd) )

v7: a fused custom-DVE op computes the loss chain with an on-op
accumulator.  With y = tn - s (tn = t*rs - mean*rs prescaled on DVE)
and c = clamp(y, -2, 2):
  4*kd = y^2 - relu(|y|-2)^2 = c*(2y - c)
Stats: sum(t) via bf16 ones-matmuls (PE), sum(t^2) via ACT Square with
free accumulation.  teacher is cast fp32->bf16 during its SWDGE DMA
(read 3x on-chip; 3-buffer window); student stays fp32 over HWDGE,
fully resident, so both streams run at full rate and interleave
per-sample.  scalar broadcast runs on PE (ones-row matmul) to keep the
GpSimd queue free for teacher DMA emission.  The last sample's loss is
quarter-chunked so its final KD op covers only the last-arriving 2048
columns.  The kernel is HBM-bound (~33.6 MB/core input stream).
Sharding: pure data parallel, 4 samples per core; host combines.
"""

from contextlib import ExitStack
from operator import add as _operator_add

import numpy as np

import concourse.mybir as mybir
import concourse.tile as tile
from concourse import bacc
from concourse.bass_utils import run_bass_kernel_spmd

B, C, H, W = 32, 256, 64, 64
N_CORES = 8
BPC = B // N_CORES            # samples per core
P = 128
N = C * H * W                 # 1048576 elements per sample
FD = N // P                   # 8192 free-dim per partition
CH = 4096                     # stats / loss chunk (last sample: 2048)
HF = FD // 2                  # student half-tile width
MM = 512                      # matmul free-dim block (PSUM bank width)
EPS = 1e-5
BETA = 2.0
LOSS_WEIGHT = 1.0

f32 = mybir.dt.float32
bf16 = mybir.dt.bfloat16
AF = mybir.ActivationFunctionType
OP = mybir.AluOpType
AX = mybir.AxisListType


# ---------------- custom fused DVE op: accum += 4*kd ----------------------
def _register_kd_op():
    import concourse.dve_ops as dve_ops
    from concourse.dve_ops import DveOp
    from concourse.dve_spec import (
        C2,
        Latch,
        Spec,
        Src0,
        Src1,
        Zero,
        _has_src1,
        lower,
        maxx,
        minn,
    )
    from concourse.dve_table_gen import dve_ver_for
    from concourse.dve_uop import DveOpSpec

    name = "TENSOR_KD_SMOOTHL1"
    if name in dve_ops._SUB_OPCODE_FOR_NAME:
        return next(op for op in dve_ops.OPS if op.name == name)

    # in0 = tn (= t*rs - mean*rs, prescaled), in1 = s.
    # y = tn - s ; c = clamp(y, -2, 2) ; body = c*(2y - c) = 4*kd
    # 6 ALU ops + 1 accum stage <= 8-stage DVE pipeline.
    y = Src0 - Src1
    c = maxx(minn(y, C2), Latch(Zero - C2))
    body = c * (y + y - c)

    def _ref(in0, in1, c0, c1, c2):
        yv = in0.astype(np.float32) - in1.astype(np.float32)
        cv = np.clip(yv, -c2, c2)
        b = (cv * (2.0 * yv - cv)).astype(np.float32)
        return b, b.reshape(b.shape[0], -1).sum(axis=-1, keepdims=True)

    spec = Spec(body=body, accum=_operator_add, reference=_ref)
    ver = dve_ver_for("TRN2")
    row = max(dve_ops._SUB_OPCODE_FOR_NAME.values()) + 1
    assert row < 0x20
    probe = DveOpSpec(
        name=name, opcode=row, uops=lower(spec, ver=ver), rd1_en=_has_src1(spec)
    )
    op = DveOp(name, spec, subdim=False, uops_sha={ver: probe.sha(ver)})
    dve_ops.OPS.append(op)
    dve_ops.CUSTOM_DVE_SPECS[name] = spec
    dve_ops._SUB_OPCODE_FOR_NAME[name] = row
    return op


KD_OP = _register_kd_op()


def _build_kernel(ctx: ExitStack, tc: "tile.TileContext", out_ap, teacher, stu):
    nc = tc.nc

    const_pool = ctx.enter_context(tc.tile_pool(name="const", bufs=1))
    t_pool = ctx.enter_context(tc.tile_pool(name="t", bufs=3))
    s_pool = ctx.enter_context(tc.tile_pool(name="s", bufs=BPC))
    s1_pool = ctx.enter_context(tc.tile_pool(name="s1", bufs=BPC - 1))
    dead_pool = ctx.enter_context(tc.tile_pool(name="dead", bufs=2))
    kdout_pool = ctx.enter_context(tc.tile_pool(name="kdout", bufs=1))
    tn_pool = ctx.enter_context(tc.tile_pool(name="tn", bufs=2))
    sums_pool = ctx.enter_context(tc.tile_pool(name="sums", bufs=3))
    tiny_pool = ctx.enter_context(tc.tile_pool(name="tiny", bufs=2))
    ps_t_pool = ctx.enter_context(tc.tile_pool(name="ps_t", bufs=2, space="PSUM"))
    ps_tt_pool = ctx.enter_context(tc.tile_pool(name="ps_tt", bufs=2, space="PSUM"))
    ps_sm_pool = ctx.enter_context(tc.tile_pool(name="ps_sm", bufs=2, space="PSUM"))
    ps_bc_pool = ctx.enter_context(tc.tile_pool(name="ps_bc", bufs=2, space="PSUM"))

    ones_bf = const_pool.tile([P, 1], bf16)
    nc.vector.memset(ones_bf[:], 1.0)
    ones_f32 = const_pool.tile([P, 1], f32)
    nc.vector.memset(ones_f32[:], 1.0)
    ones_row = const_pool.tile([1, P], f32)
    nc.vector.memset(ones_row[:], 1.0)
    staging = const_pool.tile([1, 8 * BPC], f32)
    nc.vector.memset(staging[:], 0.0)
    warm = const_pool.tile([1, 1], f32)
    # touch Abs_reciprocal_sqrt first so the one ACT table set that holds
    # both it and Square loads once, at kernel start, off the critical path
    nc.scalar.activation(warm[:], ones_f32[0:1, 0:1], AF.Abs_reciprocal_sqrt)

    # ---------------- input DMAs up front --------------------------------
    # teacher: SWDGE cast fp32->bf16, full-sample tiles, 3-buffer window
    # (the t3 emission waits on t0's release inside the GpSimd queue, which
    # carries nothing else).
    t_tiles = []
    for b in range(BPC):
        t_sb = t_pool.tile([P, FD], bf16, name="t")
        nc.gpsimd.dma_start(t_sb[:, 0:HF], teacher[b, :, 0:HF])
        nc.gpsimd.dma_start(t_sb[:, HF:FD], teacher[b, :, HF:FD])
        t_tiles.append(t_sb)
    # student: HWDGE fp32 half-sample tiles, fully resident.  The last
    # sample loads in quarter slices so its loss can start per-quarter.
    s_halves = []
    for b in range(BPC):
        h0 = s_pool.tile([P, HF], f32, name="sh0")
        h1 = s1_pool.tile([P, HF], f32, name="sh1")
        if b < BPC - 1:
            nc.sync.dma_start(h0[:], stu[b, :, 0:HF])
            nc.sync.dma_start(h1[:], stu[b, :, HF:FD])
        else:
            Q = HF // 2
            for q in range(2):
                nc.sync.dma_start(
                    h0[:, q * Q : (q + 1) * Q], stu[b, :, q * Q : (q + 1) * Q]
                )
            for q in range(2):
                nc.sync.dma_start(
                    h1[:, q * Q : (q + 1) * Q],
                    stu[b, :, HF + q * Q : HF + (q + 1) * Q],
                )
        s_halves.append((h0, h1))

    state = {}

    def stats(b):
        t_sb = t_tiles[b]
        ps_t = ps_t_pool.tile([1, MM], f32)
        nmm = FD // MM
        for k in range(nmm):
            nc.tensor.matmul(
                ps_t[:, :],
                ones_bf[:, :],
                t_sb[:, k * MM : (k + 1) * MM],
                start=(k == 0),
                stop=(k == nmm - 1),
            )
        # sums cols 4:8 hold the per-chunk sum(4kd) loss accumulators
        sums = sums_pool.tile([P, 8], f32)
        ps_tt = ps_tt_pool.tile([1, MM], f32)
        for ci in range(4):
            sl = slice(ci * 2048, (ci + 1) * 2048)
            dead = dead_pool.tile([P, 2048], bf16)
            nc.scalar.activation(dead[:], t_sb[:, sl], AF.Square)
            for k in range(2048 // MM):
                nc.tensor.matmul(
                    ps_tt[:, :],
                    ones_bf[:, :],
                    dead[:, k * MM : (k + 1) * MM],
                    start=(ci == 0 and k == 0),
                    stop=(ci == 3 and k == 2048 // MM - 1),
                )
        state[b] = {"ps_t": ps_t, "ps_tt": ps_tt, "sums": sums}

    def tiny(b):
        st_ = state[b]
        bb = tiny_pool.tile([1, 20], f32)
        ps_sm = ps_sm_pool.tile([1, 8], f32)
        st = bb[0:1, 3:4]
        nc.vector.reduce_sum(out=st, in_=st_["ps_t"][:, :], axis=AX.X)
        stt = bb[0:1, 4:5]
        nc.vector.reduce_sum(out=stt, in_=st_["ps_tt"][:, :], axis=AX.X)
        mean = bb[0:1, 2:3]
        nc.vector.tensor_scalar(mean, st, 1.0 / N, None, op0=OP.mult)
        e2 = bb[0:1, 5:6]
        nc.vector.tensor_scalar(e2, stt, 1.0 / N, EPS, op0=OP.mult, op1=OP.add)
        msq = bb[0:1, 6:7]
        nc.vector.tensor_tensor(msq, mean, mean, op=OP.mult)
        ve = bb[0:1, 7:8]
        nc.vector.tensor_tensor(ve, e2, msq, op=OP.subtract)
        rs = bb[0:1, 9:10]
        # rs0 ~= 1/sqrt(|ve|) -- table fn from the same ACT set as Square,
        # so the kernel needs exactly one ACT_TABLE_LOAD.
        nc.scalar.activation(rs, ve, AF.Abs_reciprocal_sqrt)
        # one Newton iteration: rs <- rs*(1.5 - 0.5*ve*rs^2)
        r2 = bb[0:1, 10:11]
        nc.vector.tensor_tensor(r2, rs, rs, op=OP.mult)
        pv = bb[0:1, 11:12]
        nc.vector.tensor_tensor(pv, r2, ve, op=OP.mult)
        hh = bb[0:1, 12:13]
        nc.vector.tensor_scalar(hh, pv, -0.5, 1.5, op0=OP.mult, op1=OP.add)
        rs_f = bb[0:1, 0:1]
        nc.vector.tensor_tensor(rs_f, rs, hh, op=OP.mult)
        # bb col0 = rs (final); col1 = -mean*rs
        mean_rs = bb[0:1, 13:14]
        nc.vector.tensor_tensor(mean_rs, mean, rs_f, op=OP.mult)
        nc.vector.tensor_scalar(bb[0:1, 1:2], mean_rs, -1.0, None, op0=OP.mult)
        # broadcast rs / -mean*rs to all partitions via a PE ones-row matmul
        ps_bc = ps_bc_pool.tile([P, 2], f32)
        nc.tensor.matmul(ps_bc[:, :], ones_row[:, :], bb[0:1, 0:2], start=True, stop=True)
        st_["bb"] = bb
        st_["ps_sm"] = ps_sm
        st_["rs_f"] = rs_f
        st_["mean"] = mean
        st_["rs_vec"] = ps_bc[:, 0:1]
        st_["nmrs_vec"] = ps_bc[:, 1:2]

    def loss(b):
        st_ = state[b]
        t_sb = t_tiles[b]
        sums = st_["sums"]
        nch = 2 if b < BPC - 1 else 4
        ch = FD // nch
        kdo = kdout_pool.tile([P, CH], bf16, name="kdo")
        for ci in range(nch):
            sl = slice(ci * ch, (ci + 1) * ch)
            s_half = s_halves[b][(ci * ch) // HF]
            soff = (ci * ch) % HF
            tn = tn_pool.tile([P, ch], bf16, name="tn")
            nc.vector.tensor_scalar(
                tn[:],
                t_sb[:, sl],
                st_["rs_vec"],
                st_["nmrs_vec"],
                op0=OP.mult,
                op1=OP.add,
            )
            nc.vector._custom_dve(
                KD_OP,
                out=kdo[:, 0:ch],
                in0=tn[:],
                in1=s_half[:, soff : soff + ch],
                imm2=BETA,
                accum_out=sums[:, 4 + ci : 5 + ci],
            )
        ps_sm = st_["ps_sm"]
        nc.tensor.matmul(
            ps_sm[:, 4 : 4 + nch],
            ones_f32[:, :],
            sums[:, 4 : 4 + nch],
            start=True,
            stop=True,
        )
        o = 8 * b
        nc.vector.reduce_sum(
            out=staging[0:1, o : o + 1], in_=ps_sm[0:1, 4 : 4 + nch], axis=AX.X
        )
        nc.vector.tensor_copy(staging[0:1, o + 1 : o + 2], st_["rs_f"])
        nc.vector.tensor_copy(staging[0:1, o + 2 : o + 3], st_["mean"])

    # software pipeline: tiny(b) before stats(b+1) so the ACT queue never
    # holds sample b's Sqrt hostage behind sample b+1's Squares.
    stats(0)
    with tc.high_priority():
        tiny(0)
    stats(1)
    loss(0)
    with tc.high_priority():
        tiny(1)
    stats(2)
    loss(1)
    with tc.high_priority():
        tiny(2)
    stats(3)
    loss(2)
    with tc.high_priority():
        tiny(3)
    loss(3)

    nc.sync.dma_start(out_ap[:, :], staging[:, :])


_CACHED = {}


def _get_nc():
    if "nc" in _CACHED:
        return _CACHED["nc"]
    nc = bacc.Bacc(
        "TRN2",
        target_bir_lowering=False,
        debug=False,
        enable_asserts=False,
        num_devices=N_CORES,
    )
    teacher = nc.dram_tensor("teacher", [BPC, P, FD], f32, kind="ExternalInput").ap()
    stu = nc.dram_tensor("stu", [BPC, P, FD], f32, kind="ExternalInput").ap()
    out = nc.dram_tensor("out", [1, 8 * BPC], f32, kind="ExternalOutput").ap()
    with tile.TileContext(nc) as tc:
        with ExitStack() as ctx:
            _build_kernel(ctx, tc, out, teacher, stu)
    nc.compile()
    _CACHED["nc"] = nc
    return nc


def _combine(parts):
    """parts: list of 8 arrays [1, 8*BPC] -> scalar loss."""
    losses = []
    for r in parts:
        r = np.asarray(r, dtype=np.float64).reshape(BPC, 8)
        losses.append(0.25 * r[:, 0])
    losses = np.concatenate(losses)
    return np.float32(LOSS_WEIGHT * losses.mean())


def run(inputs: dict, trace: bool = False):
    teacher = np.ascontiguousarray(np.asarray(inputs["teacher_feat"], dtype=np.float32))
    stu = np.ascontiguousarray(np.asarray(inputs["stu_feat"], dtype=np.float32))
    assert teacher.shape == (B, C, H, W) and stu.shape == (B, C, H, W)
    tch = teacher.reshape(N_CORES, BPC, P, FD)
    sch = stu.reshape(N_CORES, BPC, P, FD)
    in_maps = [
        {"teacher": np.ascontiguousarray(tch[i]), "stu": np.ascontiguousarray(sch[i])}
        for i in range(N_CORES)
    ]
    nc = _get_nc()
    res = run_bass_kernel_spmd(nc, in_maps, core_ids=list(range(N_CORES)), trace=trace)
    parts = [res.results[i]["out"] for i in range(N_CORES)]
    return _combine(parts), res


def kernel(**inputs) -> np.ndarray:
    out, _ = run(inputs, trace=False)
    return np.asarray(out, dtype=np.float32)


if __name__ == "__main__":
    rng = np.random.default_rng(0)
    ins = {
        "teacher_feat": rng.standard_normal((B, C, H, W), dtype=np.float32),
        "stu_feat": rng.standard_normal((B, C, H, W), dtype=np.float32),
    }
    print(kernel(**ins))


# revision 24
# speedup vs baseline: 1.1934x; 1.1934x over previous
"""KD feature-level smooth-L1 loss kernel for Trainium2 (8 NeuronCores).

Math (per batch sample b over (C,H,W) = 256*64*64 = N elements):
  t_norm = (t - mean) * rsqrt(var + eps)          # LayerNorm, no affine
  d   = |t_norm - s|
  kd  = where(d <= 2, d*d/4, d - 1)               # smooth-L1, beta=2
  out = mean_b( sum_chw(kd) )

A fused custom-DVE op computes the loss chain with an on-op
accumulator.  With y = tn - s (tn = t*rs - mean*rs prescaled on DVE)
and c = clamp(y, -2, 2):
  4*kd = y^2 - relu(|y|-2)^2 = c*(2y - c)
Stats: sum(t) via bf16 ones-matmuls (PE), sum(t^2) via ACT Square with
free accumulation.  teacher is cast fp32->bf16 during its SWDGE DMA
(read 3x on-chip; 3-buffer window); student stays fp32 over HWDGE,
fully resident, so both streams run at full rate and interleave
per-sample.  scalar broadcast runs on PE (ones-row matmul) to keep the
GpSimd queue free for teacher DMA emission.  The last sample's loss is
quarter-chunked so its final KD op covers only the last-arriving 2048
columns.  The kernel is HBM-bound (~33.6 MB/core input stream).
Sharding: pure data parallel, 4 samples per core; host combines.
"""

from contextlib import ExitStack
from operator import add as _operator_add

import numpy as np

import concourse.mybir as mybir
import concourse.tile as tile
from concourse import bacc
from concourse.bass_utils import run_bass_kernel_spmd

B, C, H, W = 32, 256, 64, 64
N_CORES = 8
BPC = B // N_CORES            # samples per core
P = 128
N = C * H * W                 # 1048576 elements per sample
FD = N // P                   # 8192 free-dim per partition
CH = 4096                     # loss chunk (last sample: 2048)
HF = FD // 2                  # student half-tile width
MM = 512                      # matmul free-dim block (PSUM bank width)
EPS = 1e-5
BETA = 2.0
LOSS_WEIGHT = 1.0

f32 = mybir.dt.float32
bf16 = mybir.dt.bfloat16
AF = mybir.ActivationFunctionType
OP = mybir.AluOpType
AX = mybir.AxisListType


# ---------------- custom fused DVE op: accum += 4*kd ----------------------
def _register_kd_op():
    import concourse.dve_ops as dve_ops
    from concourse.dve_ops import DveOp
    from concourse.dve_spec import (
        C2,
        Latch,
        Spec,
        Src0,
        Src1,
        Zero,
        _has_src1,
        lower,
        maxx,
        minn,
    )
    from concourse.dve_table_gen import dve_ver_for
    from concourse.dve_uop import DveOpSpec

    name = "TENSOR_KD_SMOOTHL1"
    if name in dve_ops._SUB_OPCODE_FOR_NAME:
        return next(op for op in dve_ops.OPS if op.name == name)

    # in0 = tn (= t*rs - mean*rs, prescaled), in1 = s.
    # y = tn - s ; c = clamp(y, -2, 2) ; body = c*(2y - c) = 4*kd
    # 6 ALU ops + 1 accum stage <= 8-stage DVE pipeline.
    y = Src0 - Src1
    c = maxx(minn(y, C2), Latch(Zero - C2))
    body = c * (y + y - c)

    def _ref(in0, in1, c0, c1, c2):
        yv = in0.astype(np.float32) - in1.astype(np.float32)
        cv = np.clip(yv, -c2, c2)
        b = (cv * (2.0 * yv - cv)).astype(np.float32)
        return b, b.reshape(b.shape[0], -1).sum(axis=-1, keepdims=True)

    spec = Spec(body=body, accum=_operator_add, reference=_ref)
    ver = dve_ver_for("TRN2")
    row = max(dve_ops._SUB_OPCODE_FOR_NAME.values()) + 1
    assert row < 0x20
    probe = DveOpSpec(
        name=name, opcode=row, uops=lower(spec, ver=ver), rd1_en=_has_src1(spec)
    )
    op = DveOp(name, spec, subdim=False, uops_sha={ver: probe.sha(ver)})
    dve_ops.OPS.append(op)
    dve_ops.CUSTOM_DVE_SPECS[name] = spec
    dve_ops._SUB_OPCODE_FOR_NAME[name] = row
    return op


KD_OP = _register_kd_op()


def _build_kernel(ctx: ExitStack, tc: "tile.TileContext", out_ap, teacher, stu):
    nc = tc.nc

    const_pool = ctx.enter_context(tc.tile_pool(name="const", bufs=1))
    t_pool = ctx.enter_context(tc.tile_pool(name="t", bufs=3))
    s_pool = ctx.enter_context(tc.tile_pool(name="s", bufs=BPC))
    dead_pool = ctx.enter_context(tc.tile_pool(name="dead", bufs=1))
    kdout_pool = ctx.enter_context(tc.tile_pool(name="kdout", bufs=1))
    tn_pool = ctx.enter_context(tc.tile_pool(name="tn", bufs=2))
    sums_pool = ctx.enter_context(tc.tile_pool(name="sums", bufs=3))
    tiny_pool = ctx.enter_context(tc.tile_pool(name="tiny", bufs=2))
    ps_t_pool = ctx.enter_context(tc.tile_pool(name="ps_t", bufs=3, space="PSUM"))
    ps_sm_pool = ctx.enter_context(tc.tile_pool(name="ps_sm", bufs=2, space="PSUM"))
    ps_bc_pool = ctx.enter_context(tc.tile_pool(name="ps_bc", bufs=2, space="PSUM"))

    ones_bf = const_pool.tile([P, 1], bf16)
    nc.vector.memset(ones_bf[:], 1.0)
    ones_f32 = const_pool.tile([P, 1], f32)
    nc.vector.memset(ones_f32[:], 1.0)
    ones_row = const_pool.tile([1, P], f32)
    nc.vector.memset(ones_row[:], 1.0)
    staging = const_pool.tile([1, 8 * BPC], f32)
    nc.vector.memset(staging[:], 0.0)
    warm = const_pool.tile([1, 1], f32)
    # touch Sqrt first so its ACT table set (which also holds Square) is
    # loaded once at kernel start, off the per-sample critical path
    nc.scalar.activation(warm[:], ones_f32[0:1, 0:1], AF.Sqrt)

    # ---------------- input DMAs up front --------------------------------
    # teacher: SWDGE cast fp32->bf16, full-sample tiles, 3-buffer window
    # (the t3 emission waits on t0's release inside the GpSimd queue, which
    # carries nothing else).
    t_tiles = []
    for b in range(BPC):
        t_sb = t_pool.tile([P, FD], bf16, name="t")
        nc.gpsimd.dma_start(t_sb[:, 0:HF], teacher[b, :, 0:HF])
        nc.gpsimd.dma_start(t_sb[:, HF:FD], teacher[b, :, HF:FD])
        t_tiles.append(t_sb)
    # student: HWDGE fp32 half-sample tiles, fully resident.  The last
    # sample loads in quarter slices so its loss can start per-quarter.
    s_halves = []
    for b in range(BPC):
        h0 = s_pool.tile([P, HF], f32, name="sh0")
        h1 = s_pool.tile([P, HF], f32, name="sh1")
        if b < BPC - 1:
            nc.sync.dma_start(h0[:], stu[b, :, 0:HF])
            nc.sync.dma_start(h1[:], stu[b, :, HF:FD])
        else:
            Q = HF // 2
            for q in range(2):
                nc.sync.dma_start(
                    h0[:, q * Q : (q + 1) * Q], stu[b, :, q * Q : (q + 1) * Q]
                )
            for q in range(2):
                nc.sync.dma_start(
                    h1[:, q * Q : (q + 1) * Q],
                    stu[b, :, HF + q * Q : HF + (q + 1) * Q],
                )
        s_halves.append((h0, h1))

    state = {}

    def stats(b):
        t_sb = t_tiles[b]
        ps_t = ps_t_pool.tile([1, MM], f32)
        nmm = FD // MM
        for k in range(nmm):
            nc.tensor.matmul(
                ps_t[:, :],
                ones_bf[:, :],
                t_sb[:, k * MM : (k + 1) * MM],
                start=(k == 0),
                stop=(k == nmm - 1),
            )
        # sums cols 0:4 = per-chunk sum(t^2) accums, 4:8 = sum(4kd) chunks
        sums = sums_pool.tile([P, 8], f32)
        for ci in range(4):
            sl = slice(ci * 2048, (ci + 1) * 2048)
            dead = dead_pool.tile([P, 2048], bf16)
            nc.scalar.activation(
                dead[:], t_sb[:, sl], AF.Square, accum_out=sums[:, ci : ci + 1]
            )
        state[b] = {"ps_t": ps_t, "sums": sums}

    def tiny(b):
        st_ = state[b]
        bb = tiny_pool.tile([1, 20], f32)
        ps_sm = ps_sm_pool.tile([1, 8], f32)
        nc.tensor.matmul(
            ps_sm[:, 0:4], ones_f32[:, :], st_["sums"][:, 0:4], start=True, stop=True
        )
        st = bb[0:1, 3:4]
        nc.vector.reduce_sum(out=st, in_=st_["ps_t"][:, :], axis=AX.X)
        stt = bb[0:1, 4:5]
        nc.vector.reduce_sum(out=stt, in_=ps_sm[0:1, 0:4], axis=AX.X)
        mean = bb[0:1, 2:3]
        nc.vector.tensor_scalar(mean, st, 1.0 / N, None, op0=OP.mult)
        e2 = bb[0:1, 5:6]
        nc.vector.tensor_scalar(e2, stt, 1.0 / N, EPS, op0=OP.mult, op1=OP.add)
        msq = bb[0:1, 6:7]
        nc.vector.tensor_tensor(msq, mean, mean, op=OP.mult)
        ve = bb[0:1, 7:8]
        nc.vector.tensor_tensor(ve, e2, msq, op=OP.subtract)
        inv_ve = bb[0:1, 8:9]
        nc.vector.reciprocal(inv_ve, ve)
        rs = bb[0:1, 9:10]
        nc.scalar.activation(rs, inv_ve, AF.Sqrt)  # rs0 ~= 1/sqrt(ve) (table)
        # one Newton iteration: rs <- rs*(1.5 - 0.5*ve*rs^2)
        r2 = bb[0:1, 10:11]
        nc.vector.tensor_tensor(r2, rs, rs, op=OP.mult)
        pv = bb[0:1, 11:12]
        nc.vector.tensor_tensor(pv, r2, ve, op=OP.mult)
        hh = bb[0:1, 12:13]
        nc.vector.tensor_scalar(hh, pv, -0.5, 1.5, op0=OP.mult, op1=OP.add)
        rs_f = bb[0:1, 0:1]
        nc.vector.tensor_tensor(rs_f, rs, hh, op=OP.mult)
        # bb col0 = rs (final); col1 = -mean*rs
        mean_rs = bb[0:1, 13:14]
        nc.vector.tensor_tensor(mean_rs, mean, rs_f, op=OP.mult)
        nc.vector.tensor_scalar(bb[0:1, 1:2], mean_rs, -1.0, None, op0=OP.mult)
        # broadcast rs / -mean*rs to all partitions via a PE ones-row matmul
        ps_bc = ps_bc_pool.tile([P, 2], f32)
        nc.tensor.matmul(
            ps_bc[:, :], ones_row[:, :], bb[0:1, 0:2], start=True, stop=True
        )
        st_["bb"] = bb
        st_["ps_sm"] = ps_sm
        st_["rs_f"] = rs_f
        st_["mean"] = mean
        st_["rs_vec"] = ps_bc[:, 0:1]
        st_["nmrs_vec"] = ps_bc[:, 1:2]

    def loss(b):
        st_ = state[b]
        t_sb = t_tiles[b]
        sums = st_["sums"]
        nch = 2 if b < BPC - 1 else 4
        ch = FD // nch
        kdo = kdout_pool.tile([P, CH], bf16, name="kdo")
        for ci in range(nch):
            sl = slice(ci * ch, (ci + 1) * ch)
            s_half = s_halves[b][(ci * ch) // HF]
            soff = (ci * ch) % HF
            tn = tn_pool.tile([P, ch], bf16, name="tn")
            nc.vector.tensor_scalar(
                tn[:],
                t_sb[:, sl],
                st_["rs_vec"],
                st_["nmrs_vec"],
                op0=OP.mult,
                op1=OP.add,
            )
            nc.vector._custom_dve(
                KD_OP,
                out=kdo[:, 0:ch],
                in0=tn[:],
                in1=s_half[:, soff : soff + ch],
                imm2=BETA,
                accum_out=sums[:, 4 + ci : 5 + ci],
            )
        ps_sm = st_["ps_sm"]
        nc.tensor.matmul(
            ps_sm[:, 4 : 4 + nch],
            ones_f32[:, :],
            sums[:, 4 : 4 + nch],
            start=True,
            stop=True,
        )
        o = 8 * b
        nc.vector.reduce_sum(
            out=staging[0:1, o : o + 1], in_=ps_sm[0:1, 4 : 4 + nch], axis=AX.X
        )
        nc.vector.tensor_copy(staging[0:1, o + 1 : o + 2], st_["rs_f"])
        nc.vector.tensor_copy(staging[0:1, o + 2 : o + 3], st_["mean"])

    # software pipeline: tiny(b) before stats(b+1) so the ACT queue never
    # holds sample b's Sqrt hostage behind sample b+1's Squares.
    stats(0)
    tiny(0)
    stats(1)
    loss(0)
    tiny(1)
    stats(2)
    loss(1)
    tiny(2)
    stats(3)
    loss(2)
    tiny(3)
    loss(3)

    nc.sync.dma_start(out_ap[:, :], staging[:, :])


_CACHED = {}


def _get_nc():
    if "nc" in _CACHED:
        return _CACHED["nc"]
    nc = bacc.Bacc(
        "TRN2",
        target_bir_lowering=False,
        debug=False,
        enable_asserts=False,
        num_devices=N_CORES,
    )
    teacher = nc.dram_tensor("teacher", [BPC, P, FD], f32, kind="ExternalInput").ap()
    stu = nc.dram_tensor("stu", [BPC, P, FD], f32, kind="ExternalInput").ap()
    out = nc.dram_tensor("out", [1, 8 * BPC], f32, kind="ExternalOutput").ap()
    with tile.TileContext(nc) as tc:
        with ExitStack() as ctx:
            _build_kernel(ctx, tc, out, teacher, stu)
    nc.compile()
    _CACHED["nc"] = nc
    return nc


def _combine(parts):
    """parts: list of 8 arrays [1, 8*BPC] -> scalar loss."""
    losses = []
    for r in parts:
        r = np.asarray(r, dtype=np.float64).reshape(BPC, 8)
        losses.append(0.25 * r[:, 0])
    losses = np.concatenate(losses)
    return np.float32(LOSS_WEIGHT * losses.mean())


def run(inputs: dict, trace: bool = False):
    teacher = np.ascontiguousarray(np.asarray(inputs["teacher_feat"], dtype=np.float32))
    stu = np.ascontiguousarray(np.asarray(inputs["stu_feat"], dtype=np.float32))
    assert teacher.shape == (B, C, H, W) and stu.shape == (B, C, H, W)
    tch = teacher.reshape(N_CORES, BPC, P, FD)
    sch = stu.reshape(N_CORES, BPC, P, FD)
    in_maps = [
        {"teacher": np.ascontiguousarray(tch[i]), "stu": np.ascontiguousarray(sch[i])}
        for i in range(N_CORES)
    ]
    nc = _get_nc()
    res = run_bass_kernel_spmd(nc, in_maps, core_ids=list(range(N_CORES)), trace=trace)
    parts = [res.results[i]["out"] for i in range(N_CORES)]
    return _combine(parts), res


def kernel(**inputs) -> np.ndarray:
    out, _ = run(inputs, trace=False)
    return np.asarray(out, dtype=np.float32)


if __name__ == "__main__":
    rng = np.random.default_rng(0)
    ins = {
        "teacher_feat": rng.standard_normal((B, C, H, W), dtype=np.float32),
        "stu_feat": rng.standard_normal((B, C, H, W), dtype=np.float32),
    }
    print(kernel(**ins))
